# revision 13
# baseline (speedup 1.0000x reference)
"""Expert-parallel batched-expert FFN kernel for Trainium2 (8 NeuronCores).

Reference computation (per expert e):
    y = relu(x[e] @ fc1_w[e] + fc1_b[e]) @ fc2_w[e] + fc2_b[e]

Sharding: E=8 experts, one expert per core (expert parallel, no collectives).

Per-core algorithm (T=2048 tokens, D=1024, H=4096):
  - x is fed to the device as fp16 and transposed upfront on the PE
    (DMA transpose only handles 2-byte dtypes) into xT [D, T] chunks.
  - FC1 produces yT [H, T] so FC2 can consume it as the stationary operand
    directly; both weight matrices stream (once) from DRAM in natural
    row-major layout on the scalar-engine HWDGE ring; x/out use the sync
    ring so the streams don't serialize behind each other.
  - Stream over H in blocks of 512; FC2 accumulates each block's 4 k-tiles
    in PSUM, then a DVE add folds the partial into the fp32 SBUF
    accumulator (bias b2 is folded into the first add).
  - Matmul operands are fp16 (m10): inputs round to ~2^-11 relative; all
    accumulation is fp32 in PSUM / SBUF.  Measured end-to-end L2 relative
    error vs the fp32 reference is ~6e-4.
  - Dependency-free warm-up matmuls at t=0 keep the PE clock gate (HAM) at
    8/8 through the DMA-bound ramp.
"""

from contextlib import ExitStack

import numpy as np

import concourse.bass as bass
import concourse.bacc as bacc
import concourse.mybir as mybir
import concourse.tile as tile
from concourse.bass_utils import run_bass_kernel_spmd
from concourse.masks import make_identity

E, T, D, H = 8, 2048, 1024, 4096
NCORES = 8
HB = 512           # h per stream block
FP = mybir.dt.float32
FP16 = mybir.dt.float16
RELU = mybir.ActivationFunctionType.Relu

N_BLK = H // HB                # 8
N_HI = HB // 128               # 4  h-tiles per block
N_KI = D // 128                # 8  k-tiles for FC1
N_TI = T // 128                # 16 token tiles
N_DC = D // 512                # 2  512-col chunks of D
N_C4 = T // 512                # 4  512-token chunks


def _emit_kernel(tc, out, x, w1, b1, w2, b2):
    nc = tc.nc
    with ExitStack() as ctx:
        singles = ctx.enter_context(tc.tile_pool(name="singles", bufs=1))
        xload = ctx.enter_context(tc.tile_pool(name="xload", bufs=3))
        xt_pool = ctx.enter_context(tc.tile_pool(name="xt", bufs=1))
        yt_pool = ctx.enter_context(tc.tile_pool(name="yt", bufs=N_HI))
        acc_pool = ctx.enter_context(tc.tile_pool(name="acc", bufs=1))
        w1_pool = ctx.enter_context(tc.tile_pool(name="w1", bufs=8))
        w2_pool = ctx.enter_context(tc.tile_pool(name="w2", bufs=16))
        psum = ctx.enter_context(tc.tile_pool(name="psum", bufs=4, space="PSUM"))

        ident = singles.tile([128, 128], FP16)
        make_identity(nc, ident)

        # b1 [1, H] -> [128, H//128] with [p, hi] = b1[hi*128 + p]
        b1t = singles.tile([128, H // 128], FP)
        nc.scalar.dma_start(out=b1t, in_=b1.rearrange("o (h p) -> (o p) h", p=128))

        # b2 [1, D] broadcast across partitions -> [128, D]
        b2b = singles.tile([128, D], FP)
        b2_bcast = bass.AP(tensor=b2.tensor, offset=b2.offset,
                           ap=[[0, 128]] + [list(b2.ap[-1])])
        nc.scalar.dma_start(out=b2b, in_=b2_bcast)

        # w1 viewed so a [p, k, m] DMA gives lhsT tiles: [d%128, d//128, h]
        w1v = w1.rearrange("(k p) h -> p k h", p=128)

        # HAM warm-up / keep-warm: dependency-free PE work (zeros tile via
        # DVE memset, ready almost immediately) so the clock gate reaches
        # 8/8 before the real matmuls and stays there through the
        # DMA-bound ramp.
        wtile = singles.tile([128, 128], FP16)
        nc.vector.memset(wtile, 0.0)
        _wu = [0]

        def emit_warm(n):
            for _ in range(n):
                pt = psum.tile([128, 128], FP16, tag="psA",
                               name=f"wu{_wu[0]}")
                _wu[0] += 1
                nc.tensor.transpose(out=pt, in_=wtile, identity=wtile)

        emit_warm(64)

        # xT[k][c4] = x[c4-chunk, k-tile].T
        xT = [[xt_pool.tile([128, 512], FP16, tag=f"xt{k}_{c4}",
                            name=f"xT{k}_{c4}")
               for c4 in range(N_C4)] for k in range(N_KI)]

        def emit_xpose(c4):
            for col in range(4):
                ti = c4 * 4 + col
                xs = xload.tile([128, D], FP16, tag="xload", name=f"xs{ti}")
                nc.sync.dma_start(out=xs, in_=x[ti * 128:(ti + 1) * 128, :])
                for k in range(N_KI):
                    pt = psum.tile([128, 128], FP16, tag="psA",
                                   name=f"psx{ti}_{k}")
                    nc.tensor.transpose(out=pt,
                                        in_=xs[:, k * 128:(k + 1) * 128],
                                        identity=ident)
                    nc.vector.tensor_copy(
                        xT[k][c4][:, col * 128:(col + 1) * 128], pt)

        for c4 in range(N_C4):
            emit_xpose(c4)

        accs = [[acc_pool.tile([128, 512], FP, tag=f"acc{ti}_{dc}",
                               name=f"acc{ti}_{dc}")
                 for dc in range(N_DC)] for ti in range(N_TI)]

        for b in range(N_BLK):
            if b in (1, 2):
                emit_warm(16)

            # ---- FC1: yT block [HB, T] = relu(w1.T @ xT + b1) ----
            w1p = []
            for hi in range(N_HI):
                h_abs = b * N_HI + hi
                wp = w1_pool.tile([128, N_KI, 128], FP16, tag="w1",
                                  name=f"w1p{b}_{hi}")
                nc.scalar.dma_start(
                    out=wp, in_=w1v[:, :, h_abs * 128:(h_abs + 1) * 128])
                w1p.append(wp)

            yTb = [yt_pool.tile([128, T], FP16, tag="yt",
                                name=f"yT{b}_{i}")
                   for i in range(N_HI)]
            for hi in range(N_HI):
                h_abs = b * N_HI + hi
                for half in range(N_C4 // 2):
                    pts = [psum.tile([128, 512], FP, tag="psA",
                                     name=f"psfc1_{b}_{hi}_{half}_{t}")
                           for t in range(2)]
                    for ki in range(N_KI):
                        for tch in range(2):
                            nc.tensor.matmul(
                                pts[tch],
                                lhsT=w1p[hi][:, ki, :],
                                rhs=xT[ki][half * 2 + tch],
                                start=(ki == 0), stop=(ki == N_KI - 1))
                    for tch in range(2):
                        c4 = half * 2 + tch
                        nc.scalar.activation(
                            out=yTb[hi][:, c4 * 512:(c4 + 1) * 512],
                            in_=pts[tch],
                            func=RELU, bias=b1t[:, h_abs:h_abs + 1], scale=1.0)

            if b == 0:
                emit_warm(16)

            # ---- FC2 partial: acc += yTb.T @ w2[block] ----
            w2t = [[None] * N_DC for _ in range(N_HI)]
            for hk in range(N_HI):
                h_abs = b * N_HI + hk
                for dc in range(N_DC):
                    wt = w2_pool.tile([128, 512], FP16, tag="w2",
                                      name=f"w2t{b}_{hk}_{dc}")
                    nc.scalar.dma_start(
                        out=wt,
                        in_=w2[h_abs * 128:(h_abs + 1) * 128,
                               dc * 512:(dc + 1) * 512])
                    w2t[hk][dc] = wt

            for ti in range(N_TI):
                pts = [psum.tile([128, 512], FP, tag="psB",
                                 name=f"psfc2_{b}_{ti}_{d}")
                       for d in range(N_DC)]
                for hk in range(N_HI):
                    for dc in range(N_DC):
                        nc.tensor.matmul(
                            pts[dc],
                            lhsT=yTb[hk][:, ti * 128:(ti + 1) * 128],
                            rhs=w2t[hk][dc],
                            start=(hk == 0), stop=(hk == N_HI - 1))
                for dc in range(N_DC):
                    if b == 0:
                        nc.vector.tensor_add(
                            accs[ti][dc], pts[dc],
                            b2b[:, dc * 512:(dc + 1) * 512])
                    else:
                        nc.vector.tensor_add(
                            accs[ti][dc], accs[ti][dc], pts[dc])

        # ---- store ----
        for ti in range(N_TI):
            for dc in range(N_DC):
                nc.sync.dma_start(
                    out=out[ti * 128:(ti + 1) * 128,
                            dc * 512:(dc + 1) * 512],
                    in_=accs[ti][dc])


def build_module():
    nc = bacc.Bacc("TRN2", target_bir_lowering=False, debug=False)
    x = nc.dram_tensor("x", [T, D], FP16, kind="ExternalInput").ap()
    w1 = nc.dram_tensor("fc1_w", [D, H], FP16, kind="ExternalInput").ap()
    b1 = nc.dram_tensor("fc1_b", [1, H], FP, kind="ExternalInput").ap()
    w2 = nc.dram_tensor("fc2_w", [H, D], FP16, kind="ExternalInput").ap()
    b2 = nc.dram_tensor("fc2_b", [1, D], FP, kind="ExternalInput").ap()
    out = nc.dram_tensor("out", [T, D], FP, kind="ExternalOutput").ap()
    with tile.TileContext(nc) as tc:
        _emit_kernel(tc, out, x, w1, b1, w2, b2)
    nc.compile()
    return nc


_CACHED = None


def kernel(x, fc1_w, fc1_b, fc2_w, fc2_b, _trace=False, _trace_cores=None):
    global _CACHED
    if _CACHED is None:
        _CACHED = build_module()
    nc = _CACHED

    x = np.ascontiguousarray(np.asarray(x, dtype=np.float32).astype(np.float16))
    fc1_w = np.ascontiguousarray(
        np.asarray(fc1_w, dtype=np.float32).astype(np.float16))
    fc1_b = np.ascontiguousarray(np.asarray(fc1_b, dtype=np.float32))
    fc2_w = np.ascontiguousarray(
        np.asarray(fc2_w, dtype=np.float32).astype(np.float16))
    fc2_b = np.ascontiguousarray(np.asarray(fc2_b, dtype=np.float32))

    in_maps = [
        {
            "x": x[e],
            "fc1_w": fc1_w[e],
            "fc1_b": fc1_b[e],
            "fc2_w": fc2_w[e],
            "fc2_b": fc2_b[e],
        }
        for e in range(E)
    ]
    kw = {}
    if _trace:
        kw = dict(trace=True,
                  trace_cores=_trace_cores if _trace_cores is not None else [0])
    res = run_bass_kernel_spmd(nc, in_maps, core_ids=list(range(NCORES)), **kw)
    out = np.stack([res.results[e]["out"] for e in range(E)], axis=0)
    if _trace:
        return out, res
    return out
